# revision 20
# baseline (speedup 1.0000x reference)
"""ESPnet-style attention decoder (nn_Decoder) on 8 Trainium2 NeuronCores.

Strategy (8-way SPMD, one chip):
- Recurrence is 8-way tensor-parallel over the 4096 LSTM gate dim (512
  gates/core, grouped as 128 of each of i/f/g/o via a host-side row
  permutation), batch-parallel attention (4 sequences/core).
- TWO collectives per decode step (down from three): AllGather(att_c)
  and a fused AllGather carrying z0(t) ++ z1(t-1).  LSTM1 trails one
  step and its 17 matmuls + cell fill the att-AllGather wait window.
- e / att_c rows are computed with zero-padded block-diagonal selector
  operands (sel/selw) so M=4 matmuls write per-sequence rows directly
  into one [4, T] PSUM tile - no block-diagonal extraction DMAs.
- Softmax skips the max-subtraction (|2e| is small); the x2 scale and
  the hlens mask ride inside the e matmul (mask/2 lives in pre_enc row
  320 against a pinned dec column of 1.0), so Exp reads PSUM directly.
  Dummy ops prewarm the Exp/Sigmoid tables off the critical path and
  the 1/sum normalization is deferred into att_c's PSUM->SBUF copy.
- Gate pre-activations accumulate fully in PSUM: x0/bias terms are
  injected via identity matmuls and the LSTM cells read PSUM directly
  (gates reordered host-side to i|f|o|g for one wide sigmoid).
- All recurrence/attention/logits matmuls and collective payloads are
  bf16; z-state cell outputs are DMAd untransposed into the fused AG
  and the gathered [512,128] block is transposed back in one XBAR
  transpose-DMA.
- The embedding x-contribution X0 = ey @ W_ih0[:, :1024]^T + biases is
  precomputed for all steps as one parallel matmul (bf16).
- Final phase: logits are output-dim-parallel (1250 vocab cols/core,
  bf16 matmuls, bias via a ones-row matmul); per-row (local-max,
  local-sumexp, label-logit) partials are returned and the host merges
  them into loss/acc/ppl.
"""
import os
import sys

sys.path.insert(0, "/opt/trn_rl_repo")

import numpy as np
import ml_dtypes

import concourse.bass as bass
import concourse.tile as tile
from concourse import bacc, mybir
from concourse import bass_utils

f32 = mybir.dt.float32
f32r = mybir.dt.float32r
bf16 = mybir.dt.bfloat16
FT = mybir.ActivationFunctionType
OP = mybir.AluOpType
AX = mybir.AxisListType

NC = 8
B, T, EPROJS = 32, 512, 512
DUNITS, ODIM, ATT_DIM = 1024, 10000, 320
APAD = 384            # ATT_DIM padded to 3*128
L = 128
S = int(os.environ.get("DEC_STEPS", L + 1))   # decode steps (129)
SOS = EOS = ODIM - 1
BL = B // NC          # sequences per core (4)
GS = 4 * DUNITS // NC  # gate slice per core (512)
ZS = DUNITS // NC     # hidden slice per core (128)
OS = ODIM // NC       # vocab slice per core (1250)

_BUILD_CACHE = {}
_SKIP = set(os.environ.get("KSKIP", "").split(","))


def _cell(nc, W, g_ps, c_sb, tag):
    """LSTM cell on a [32, 512] gate slice (i|f|o|g blocks of 128).
    Reads gates from PSUM, updates c_sb in place, returns z [32,128]."""
    sif = W.tile([B, 384], f32, tag=tag + "sif")
    nc.scalar.activation(out=sif[:], in_=g_ps[:, 0:384], func=FT.Sigmoid)
    tg = W.tile([B, ZS], f32, tag=tag + "tg")
    nc.scalar.activation(out=tg[:], in_=g_ps[:, 384:512], func=FT.Tanh)
    t1 = W.tile([B, ZS], f32, tag=tag + "t1")
    nc.vector.tensor_mul(out=t1[:], in0=sif[:, 128:256], in1=c_sb[:])
    t2 = W.tile([B, ZS], f32, tag=tag + "t2")
    nc.vector.tensor_mul(out=t2[:], in0=sif[:, 0:128], in1=tg[:])
    nc.vector.tensor_add(out=c_sb[:], in0=t1[:], in1=t2[:])
    tc_ = W.tile([B, ZS], f32, tag=tag + "tc")
    nc.scalar.activation(out=tc_[:], in_=c_sb[:], func=FT.Tanh)
    zn = W.tile([B, ZS], bf16, tag=tag + "zn")
    nc.vector.tensor_mul(out=zn[:], in0=sif[:, 256:384], in1=tc_[:])
    return zn


def build(steps):
    nrow = steps * B
    nch = (nrow + 127) // 128
    tpad = 4 * nch

    nc = bacc.Bacc("TRN2", target_bir_lowering=False, debug=False,
                   num_devices=NC)

    def din(name, shape, dt):
        return nc.dram_tensor(name, shape, dt, kind="ExternalInput")

    hs_nat = din("hs_nat", (128, BL, 4, EPROJS), bf16)
    hsT = din("hsT", (128, 4, BL * T), bf16)
    eysT = din("eysT", (128, 8, nrow), bf16)
    wih0pT = din("wih0pT", (128, 8, GS), bf16)
    x0bias = din("x0bias", (1, GS), f32)
    wencT = din("wencT", (128, 4, APAD), bf16)
    bencp = din("bencp", (128, 3), f32)
    wdecT = din("wdecT", (128, 8, APAD), bf16)
    wattT = din("wattT", (128, 4, GS), bf16)
    whh0T = din("whh0T", (128, 8, GS), bf16)
    wih1T = din("wih1T", (128, 8, GS), bf16)
    whh1T = din("whh1T", (128, 8, GS), bf16)
    bias1 = din("bias1", (1, GS), bf16)
    maskh = din("maskh", (1, BL * T), bf16)
    sel = din("sel", (B, 4 * BL), bf16)
    selw = din("selw", (BL, 4 * BL), bf16)
    woutT = din("woutT", (128, 8, OS), bf16)
    boutsl = din("boutsl", (1, OS), f32)
    labels = din("labels", (128, nch), f32)
    ident = din("ident", (128, 128), f32r)

    out_stats = nc.dram_tensor("out_stats", (128, nch, 3), f32,
                               kind="ExternalOutput")

    rg = [list(range(NC))]

    with tile.TileContext(nc) as tc:
        with tc.tile_pool(name="dram", bufs=1, space="DRAM") as DR:
            zs_dram = DR.tile([tpad, 128, 8, B], bf16, tag="zs")
            x0_dram = DR.tile([steps, B, GS], bf16, tag="x0")

            with tc.tile_pool(name="persist", bufs=1) as P:
                # ------------- persistent SBUF -------------
                hs_sb = P.tile([128, BL, 4, EPROJS], bf16)
                nc.sync.dma_start(hs_sb[:], hs_nat[:])
                wdecT_sb = P.tile([128, 8, APAD], bf16)
                nc.sync.dma_start(wdecT_sb[:], wdecT[:])
                wattT_sb = P.tile([128, 4, GS], bf16)
                nc.sync.dma_start(wattT_sb[:], wattT[:])
                whh0T_sb = P.tile([128, 8, GS], bf16)
                nc.sync.dma_start(whh0T_sb[:], whh0T[:])
                wih1T_sb = P.tile([128, 8, GS], bf16)
                nc.sync.dma_start(wih1T_sb[:], wih1T[:])
                whh1T_sb = P.tile([128, 8, GS], bf16)
                nc.sync.dma_start(whh1T_sb[:], whh1T[:])
                bias1_sb = P.tile([B, GS], bf16)
                nc.sync.dma_start(
                    bias1_sb[:],
                    bass.AP(tensor=bias1.ap().tensor, offset=0,
                            ap=[[0, B], [1, GS]]))
                sel_sb = P.tile([B, 4 * BL], bf16)
                nc.sync.dma_start(sel_sb[:], sel[:])
                selw_sb = P.tile([BL, 4 * BL], bf16)
                nc.sync.dma_start(selw_sb[:], selw[:])
                ident_sb = P.tile([128, 128], f32r)
                nc.sync.dma_start(ident_sb[:], ident[:])
                pre_encT_sb = P.tile([128, 3, BL * T], bf16)

                # z state, XBAR layout: col c*64+h*32+b = (chunk c,
                # h=0 -> z0T, h=1 -> z1T, batch b)
                zz_sb = P.tile([128, 512], bf16)
                zero0_sb = P.tile([128, 128], f32)
                nc.vector.memset(zero0_sb[:], 0.0)
                for q in range(4):
                    nc.vector.tensor_copy(
                        out=zz_sb[:, q * 128:(q + 1) * 128],
                        in_=zero0_sb[:])
                zerob_sb = P.tile([128, 256], bf16)
                for q in range(2):
                    nc.vector.tensor_copy(
                        out=zerob_sb[:, q * 128:(q + 1) * 128],
                        in_=zero0_sb[:])
                identb_sb = P.tile([B, B], bf16)
                nc.vector.tensor_copy(out=identb_sb[:],
                                      in_=ident_sb[0:B, 0:B])
                c0_sb = P.tile([B, ZS], f32)
                nc.vector.memset(c0_sb[:], 0.0)
                c1_sb = P.tile([B, ZS], f32)
                nc.vector.memset(c1_sb[:], 0.0)
                dumz_sb = P.tile([1, 1], f32)
                nc.vector.memset(dumz_sb[:], 0.0)
                dumo_sb = P.tile([1, 1], f32)
                ones_sb = P.tile([B, 1], f32)
                nc.vector.memset(ones_sb[:], 1.0)

                def zz0(kt):
                    return zz_sb[:, kt * 64: kt * 64 + 32]

                def zz1(kt):
                    return zz_sb[:, kt * 64 + 32: kt * 64 + 64]

                # zero the zs padding slots
                for tp in range(steps, tpad):
                    nc.sync.dma_start(zs_dram[tp], zerob_sb[:])

                # ------------- prologue A: pre_enc -------------
                if "pre" not in _SKIP:
                 with (
                    tc.tile_pool(name="prA", bufs=1) as PA,
                    tc.tile_pool(name="prAps", bufs=1, space="PSUM") as PAP,
                ):
                    hsT_sb = PA.tile([128, 4, BL * T], bf16, tag="hsT")
                    nc.sync.dma_start(hsT_sb[:], hsT[:])
                    wencT_sb = PA.tile([128, 4, APAD], bf16, tag="wenc")
                    nc.sync.dma_start(wencT_sb[:], wencT[:])
                    bencp_sb = PA.tile([128, 3], f32, tag="benc")
                    nc.sync.dma_start(bencp_sb[:], bencp[:])
                    for ac in range(3):
                        ps = PAP.tile([128, BL * T], f32, tag="pe")
                        for dk in range(4):
                            for ns in range(4):
                                nc.tensor.matmul(
                                    ps[:, ns * 512:(ns + 1) * 512],
                                    wencT_sb[:, dk, ac * 128:(ac + 1) * 128],
                                    hsT_sb[:, dk, ns * 512:(ns + 1) * 512],
                                    start=(dk == 0), stop=(dk == 3))
                        nc.scalar.activation(
                            out=pre_encT_sb[:, ac, :], in_=ps[:],
                            func=FT.Tanh, bias=bencp_sb[:, ac:ac + 1],
                            scale=1.0)

                # mask/2 -> pre_enc row 320 (dec col 320 is pinned to 1)
                nc.sync.dma_start(pre_encT_sb[64:65, 2, :], maskh[:])

                # ------------- prologue B: X0 precompute -------------
                if "x0" not in _SKIP:
                 with (
                    tc.tile_pool(name="prB", bufs=2) as PB,
                    tc.tile_pool(name="prB1", bufs=1) as PB1,
                    tc.tile_pool(name="prBps", bufs=2, space="PSUM") as PBP,
                ):
                    wih0pT_sb = PB1.tile([128, 8, GS], bf16, tag="wih0p")
                    nc.sync.dma_start(wih0pT_sb[:], wih0pT[:])
                    x0bias_sb = PB1.tile([128, GS], f32, tag="x0b")
                    nc.sync.dma_start(
                        x0bias_sb[:],
                        bass.AP(tensor=x0bias.ap().tensor, offset=0,
                                ap=[[0, 128], [1, GS]]))
                    x0_flat = x0_dram[:].rearrange("t b g -> (t b) g")
                    for ch in range(nch):
                        cw = min(128, nrow - ch * 128)
                        ey_t = PB.tile([128, 8, 128], bf16, tag="eych")
                        nc.sync.dma_start(
                            ey_t[:, :, :cw],
                            eysT[:, :, ch * 128: ch * 128 + cw])
                        ps = PBP.tile([128, GS], f32, tag="x0")
                        for kt in range(8):
                            nc.tensor.matmul(
                                ps[:cw, :], ey_t[:, kt, :cw],
                                wih0pT_sb[:, kt, :],
                                start=(kt == 0), stop=(kt == 7))
                        g = PB.tile([128, GS], bf16, tag="x0g")
                        nc.vector.tensor_tensor(
                            out=g[:cw, :], in0=ps[:cw, :],
                            in1=x0bias_sb[:cw, :], op=OP.add)
                        nc.sync.dma_start(
                            x0_flat[ch * 128: ch * 128 + cw, :], g[:cw, :])

                # ------------- recurrence -------------
                if "rec" not in _SKIP:
                 with (
                    tc.tile_pool(name="work", bufs=2) as W,
                    tc.tile_pool(name="ps_a", bufs=1, space="PSUM") as PSa,
                    tc.tile_pool(name="ps_sm", bufs=1, space="PSUM") as PSsm,
                    tc.tile_pool(name="ps_g0", bufs=1, space="PSUM") as PSg0,
                    tc.tile_pool(name="ps_g1", bufs=1, space="PSUM") as PSg1,
                    tc.tile_pool(name="bnc", bufs=2, space="DRAM") as BN,
                    tc.tile_pool(name="shr", bufs=2, space="DRAM") as SH,
                    tc.tile_pool(name="x0pre", bufs=2) as X0P,
                ):
                    def lstm1_trailing(t):
                        """g1(t) + cell1(t) from zz (z0(t), z1(t-1)) and
                        write the z1T slice into `dst` DRAM cols."""
                        g1_ps = PSg1.tile([B, GS], f32, tag="g1")
                        for kt in range(8):
                            nc.tensor.matmul(g1_ps[:], zz0(kt),
                                             wih1T_sb[:, kt, :],
                                             start=(kt == 0), stop=False)
                        for kt in range(8):
                            nc.tensor.matmul(g1_ps[:], zz1(kt),
                                             whh1T_sb[:, kt, :],
                                             start=False, stop=False)
                        nc.tensor.matmul(g1_ps[:], identb_sb[:],
                                         bias1_sb[:],
                                         start=False, stop=True)
                        z1n = _cell(nc, W, g1_ps, c1_sb, "c1")
                        return z1n

                    for t in range(steps):
                        x0_t = X0P.tile([B, GS], bf16, tag="x0t")
                        nc.sync.dma_start(x0_t[:], x0_dram[t])

                        # --- dec = tanh(z0 @ WdecT), all 32 seqs ---
                        a_ps = PSa.tile([B, 512], f32, tag="a")
                        for kt in range(8):
                            nc.tensor.matmul(a_ps[:, 0:APAD], zz0(kt),
                                             wdecT_sb[:, kt, :],
                                             start=(kt == 0), stop=(kt == 7))
                        dec_sb = W.tile([B, APAD], bf16, tag="dec")
                        nc.scalar.activation(out=dec_sb[:],
                                             in_=a_ps[:, 0:APAD],
                                             func=FT.Tanh)
                        nc.vector.tensor_copy(out=dec_sb[:, 320:321],
                                              in_=ones_sb[:])
                        # prewarm Exp table while PE runs (dummy op)
                        nc.scalar.activation(out=dumo_sb[:], in_=dumz_sb[:],
                                             func=FT.Exp)

                        # --- decT (zero-padded block-diag) via selector ---
                        # dT_pad[:, ac, 4j:4j+4] has only column j nonzero,
                        # so M=4 e-matmuls write row j with rows i!=j += 0.
                        dT_ps = PSsm.tile([128, 3, 4 * BL], f32, tag="dT")
                        for ac in range(3):
                            nc.tensor.matmul(
                                dT_ps[:, ac, :],
                                dec_sb[:, ac * 128:(ac + 1) * 128],
                                sel_sb[:], start=True, stop=True)
                        decT_sb = W.tile([128, 3, 4 * BL], bf16, tag="dT")
                        nc.vector.tensor_copy(out=decT_sb[:], in_=dT_ps[:])

                        # --- hh0 + x0 prefire into g0 (overlaps softmax) ---
                        g0_ps = PSg0.tile([B, GS], f32, tag="g0")
                        for kt in range(8):
                            nc.tensor.matmul(g0_ps[:], zz0(kt),
                                             whh0T_sb[:, kt, :],
                                             start=(kt == 0), stop=False)
                        nc.tensor.matmul(g0_ps[:], identb_sb[:],
                                         x0_t[:],
                                         start=False, stop=False)

                        # --- e rows (padded M=4 matmuls, no extraction) ---
                        e_ps = PSa.tile([B, 512], f32, tag="a")
                        nmm = BL * 3
                        im = 0
                        for j in range(BL):
                            for ac in range(3):
                                nc.tensor.matmul(
                                    e_ps[0:BL, :],
                                    decT_sb[:, ac, 4 * j:4 * j + 4],
                                    pre_encT_sb[:, ac, j * T:(j + 1) * T],
                                    start=(im == 0), stop=(im == nmm - 1))
                                im += 1

                        # --- softmax: w_u = exp(2*(e + mask/2)), PSUM-direct ---
                        w_u = W.tile([BL, T], bf16, tag="wu")
                        ssum = W.tile([BL, 1], f32, tag="ssum")
                        nc.scalar.activation(
                            out=w_u[:], in_=e_ps[0:BL, :], func=FT.Exp,
                            scale=2.0, accum_out=ssum[:])
                        rsum = W.tile([BL, 1], f32, tag="rsum")
                        nc.vector.reciprocal(out=rsum[:], in_=ssum[:])

                        # --- wT (padded block-diag transpose via matmul) ---
                        wT_ps = PSsm.tile([128, 4, 4 * BL], f32, tag="wT")
                        for tk in range(4):
                            nc.tensor.matmul(
                                wT_ps[:, tk, :],
                                w_u[:, tk * 128:(tk + 1) * 128],
                                selw_sb[:], start=True, stop=True)
                        wT_sb = W.tile([128, 4, 4 * BL], bf16, tag="wT")
                        nc.vector.tensor_copy(out=wT_sb[:], in_=wT_ps[:])

                        # --- att_c rows (padded M=4, unnormalized) ---
                        ac_ps = PSa.tile([B, 512], f32, tag="a")
                        im = 0
                        for j in range(BL):
                            for tk in range(4):
                                nc.tensor.matmul(
                                    ac_ps[0:BL, :],
                                    wT_sb[:, tk, 4 * j:4 * j + 4],
                                    hs_sb[:, j, tk, :],
                                    start=(im == 0), stop=(im == 15))
                                im += 1
                        # normalize into the PSUM->SBUF copy
                        ac_sb = W.tile([BL, EPROJS], bf16, tag="acsb")
                        nc.vector.tensor_scalar_mul(
                            out=ac_sb[:], in0=ac_ps[0:BL, :], scalar1=rsum[:])

                        # --- AllGather att_c -> [32, 512] ---
                        acb_in = BN.tile([BL, EPROJS], bf16, tag="acb")
                        nc.gpsimd.dma_start(acb_in[:], ac_sb[:])
                        acb_out = SH.tile([B, EPROJS], bf16, tag="acs",
                                          addr_space="Shared")
                        nc.gpsimd.collective_compute(
                            "AllGather", OP.bypass, replica_groups=rg,
                            ins=[acb_in[:]], outs=[acb_out[:]])

                        # --- LSTM1(t-1) fills the AG window ---
                        zb_in = BN.tile([64, 128], bf16, tag="zb")
                        if t > 0:
                            z1n = lstm1_trailing(t - 1)
                            nc.sync.dma_start(zb_in[32:64, :], z1n[:])
                            # prewarm sigmoid/tanh table for cell0
                            nc.scalar.activation(out=dumo_sb[:],
                                                 in_=dumz_sb[:],
                                                 func=FT.Sigmoid)
                        else:
                            nc.sync.dma_start(
                                zb_in[32:64, :], zerob_sb[0:32, 0:128])

                        # --- post-AG: attT via PE transposes ---
                        attall_sb = W.tile([B, EPROJS], bf16, tag="attall")
                        nc.gpsimd.dma_start(attall_sb[:], acb_out[:])
                        aT_ps = PSsm.tile([128, 4, B], bf16, tag="attT")
                        for dk in range(4):
                            nc.tensor.transpose(
                                aT_ps[:, dk, :],
                                attall_sb[:, dk * 128:(dk + 1) * 128],
                                identb_sb[:])
                        attT_sb = W.tile([128, 4, B], bf16, tag="attT")
                        nc.vector.tensor_copy(out=attT_sb[:], in_=aT_ps[:])
                        for dk in range(4):
                            nc.tensor.matmul(g0_ps[:], attT_sb[:, dk, :],
                                             wattT_sb[:, dk, :],
                                             start=False, stop=(dk == 3))

                        # --- cell0 -> z0 slice rows -> fused AG ---
                        z0n = _cell(nc, W, g0_ps, c0_sb, "c0")
                        nc.gpsimd.dma_start(zb_in[0:32, :], z0n[:])

                        zb_out = SH.tile([64 * NC, 128], bf16, tag="zbs",
                                         addr_space="Shared")
                        nc.gpsimd.collective_compute(
                            "AllGather", OP.bypass, replica_groups=rg,
                            ins=[zb_in[:]], outs=[zb_out[:]])
                        # XBAR transpose DMA: [512,128] -> zz [128,512]
                        nc.scalar.dma_start_transpose(zz_sb[:], zb_out[:])
                        if t > 0:
                            nc.sync.dma_start(
                                zs_dram[t - 1],
                                zz_sb[:]
                                .rearrange("k (c b2) -> k c b2", b2=64)
                                [:, :, 32:64])

                    # --- tail: LSTM1(S-1) + final z1 AllGather ---
                    z1n = lstm1_trailing(steps - 1)
                    zb2_in = BN.tile([B, 128], bf16, tag="zb2")
                    nc.sync.dma_start(zb2_in[:], z1n[:])
                    zb2_out = SH.tile([B * NC, 128], bf16, tag="zb2s",
                                      addr_space="Shared")
                    nc.gpsimd.collective_compute(
                        "AllGather", OP.bypass, replica_groups=rg,
                        ins=[zb2_in[:]], outs=[zb2_out[:]])
                    ztail = W.tile([128, 256], bf16, tag="ztail")
                    nc.scalar.dma_start_transpose(ztail[:], zb2_out[:])
                    nc.sync.dma_start(zs_dram[steps - 1], ztail[:])

            # ------------- logits + partial log-softmax -------------
            if "log" not in _SKIP:
             with (
                tc.tile_pool(name="lg", bufs=2) as LG,
                tc.tile_pool(name="lg1", bufs=1) as LG1,
                tc.tile_pool(name="lgps", bufs=2, space="PSUM") as LPS,
            ):
                woutT_sb = LG1.tile([128, 8, OS], bf16, tag="wout")
                nc.sync.dma_start(woutT_sb[:], woutT[:])
                bout_sb = LG1.tile([1, OS], f32, tag="bout")
                nc.sync.dma_start(bout_sb[:], boutsl.ap())
                onesr_sb = LG1.tile([1, 128], f32, tag="onesr")
                nc.vector.memset(onesr_sb[:], 1.0)
                lab_sb = LG1.tile([128, nch], f32, tag="lab")
                nc.sync.dma_start(lab_sb[:], labels[:])
                iota_sb = LG1.tile([128, OS], f32, tag="iota")
                nc.gpsimd.iota(iota_sb[:], pattern=[[1, OS]], base=0,
                               channel_multiplier=0,
                               allow_small_or_imprecise_dtypes=True)
                m_all = LG1.tile([128, nch], f32, tag="m")
                s_all = LG1.tile([128, nch], f32, tag="s")
                lg_all = LG1.tile([128, nch], f32, tag="lg")

                osubs = [(0, 512), (512, 512), (1024, OS - 1024)]
                for ch in range(nch):
                    zch = LG.tile([128, 8, 4, B], bf16, tag="zch")
                    nc.sync.dma_start(
                        zch[:],
                        zs_dram[4 * ch: 4 * ch + 4]
                        .rearrange("t k kt b -> k kt t b"))
                    zch_f = zch[:].rearrange("k kt t b -> k kt (t b)")
                    ps = LPS.tile([128, OS], f32, tag="lps")
                    for (o0, ow) in osubs:
                        nc.tensor.matmul(
                            ps[:, o0:o0 + ow], onesr_sb[:],
                            bout_sb[:, o0:o0 + ow],
                            start=True, stop=False)
                        for kt in range(8):
                            nc.tensor.matmul(
                                ps[:, o0:o0 + ow], zch_f[:, kt, :],
                                woutT_sb[:, kt, o0:o0 + ow],
                                start=False, stop=(kt == 7))
                    buf = LG.tile([128, OS], bf16, tag="lbuf")
                    nc.vector.tensor_copy(out=buf[:], in_=ps[:])
                    negm = LG.tile([128, 1], f32, tag="lnegm")
                    nc.vector.tensor_reduce(out=negm[:], in_=buf[:],
                                            op=OP.max, axis=AX.X, negate=True)
                    nc.vector.tensor_scalar_mul(
                        out=m_all[:, ch:ch + 1], in0=negm[:], scalar1=-1.0)
                    if "lmask" not in _SKIP:
                        mask = LG.tile([128, OS], bf16, tag="lmask")
                        nc.vector.tensor_scalar(
                            out=mask[:], in0=iota_sb[:],
                            scalar1=lab_sb[:, ch:ch + 1], scalar2=None,
                            op0=OP.is_equal)
                        prod = LG.tile([128, OS], bf16, tag="lprod")
                        nc.vector.tensor_mul(out=prod[:], in0=buf[:],
                                             in1=mask[:])
                        nc.vector.tensor_reduce(
                            out=lg_all[:, ch:ch + 1], in_=prod[:],
                            op=OP.add, axis=AX.X)
                    if "lexp" not in _SKIP:
                        buf2 = LG.tile([128, OS], bf16, tag="lbuf2")
                        nc.scalar.activation(
                            out=buf2[:], in_=buf[:], func=FT.Exp,
                            bias=negm[:], scale=1.0,
                            accum_out=s_all[:, ch:ch + 1])

                nc.sync.dma_start(out_stats[:, :, 0], m_all[:])
                nc.sync.dma_start(
                    out_stats[:, :, 1],
                    m_all[:] if "lexp" in _SKIP else s_all[:])
                nc.sync.dma_start(
                    out_stats[:, :, 2],
                    m_all[:] if "lmask" in _SKIP else lg_all[:])

    nc.finalize()
    return nc


# ---------------------------------------------------------------------------
# host side
# ---------------------------------------------------------------------------

def _prep_inputs(hs_pad, hlens, ys_pad, embed_w, Wenc, benc, Wdec,
                 W_ih0, W_hh0, b_ih0, b_hh0, W_ih1, W_hh1, b_ih1, b_hh1,
                 Wout, bout, steps):
    """Shard + pack all inputs into per-core in_maps (pure data movement)."""
    f = np.float32
    hs_pad = np.asarray(hs_pad, f)
    ys_pad = np.asarray(ys_pad)
    ys_in = np.concatenate(
        [np.full((B, 1), SOS, ys_pad.dtype), ys_pad], axis=1)[:, :steps]
    ys_out = np.concatenate(
        [ys_pad, np.full((B, 1), EOS, ys_pad.dtype)], axis=1)[:, :steps]

    # gate permutation: core c's rows = 128 each of i/f/o/g
    perm = np.concatenate(
        [g * DUNITS + c * ZS + np.arange(ZS)
         for c in range(NC) for g in (0, 1, 3, 2)])

    eys = np.asarray(embed_w, f)[ys_in]                  # [B, steps, 1024]
    eysT = np.ascontiguousarray(
        eys.transpose(2, 1, 0).reshape(DUNITS, steps * B))
    eysT = np.ascontiguousarray(
        eysT.reshape(8, 128, -1).transpose(1, 0, 2)).astype(
            ml_dtypes.bfloat16)                          # [128, 8, rows]

    def kpack(M, dt=f):
        """[K, N] -> [128, K//128, N]"""
        K = M.shape[0]
        return np.ascontiguousarray(
            M.reshape(K // 128, 128, -1).transpose(1, 0, 2)).astype(dt)

    W_ih0 = np.asarray(W_ih0, f)[perm]
    W_hh0 = np.asarray(W_hh0, f)[perm]
    W_ih1 = np.asarray(W_ih1, f)[perm]
    W_hh1 = np.asarray(W_hh1, f)[perm]
    bias0 = (np.asarray(b_ih0, f) + np.asarray(b_hh0, f))[perm]
    bias1v = (np.asarray(b_ih1, f) + np.asarray(b_hh1, f))[perm]

    wencp = np.zeros((APAD, EPROJS), f)
    wencp[:ATT_DIM] = np.asarray(Wenc, f)
    bencpv = np.zeros((3, 128), f)
    bencpv.reshape(-1)[:ATT_DIM] = np.asarray(benc, f)
    wdecp = np.zeros((APAD, DUNITS), f)
    wdecp[:ATT_DIM] = np.asarray(Wdec, f)

    wencT = kpack(wencp.T)                      # [128, 4, 384]
    wdecT = kpack(wdecp.T, ml_dtypes.bfloat16)  # [128, 8, 384]
    identv = np.eye(128, dtype=f)

    Wout = np.asarray(Wout, f)
    bout_v = np.asarray(bout, f)

    ys_out_flat = ys_out.T.reshape(-1)          # row r = t*B + b
    nrow = steps * B
    nch = (nrow + 127) // 128

    in_maps = []
    for c in range(NC):
        sl = slice(GS * c, GS * (c + 1))
        seqs = slice(BL * c, BL * (c + 1))
        hs_c = hs_pad[seqs]                     # [4, 512, 512]
        hs_nat = np.ascontiguousarray(
            hs_c.reshape(BL, 4, 128, EPROJS).transpose(2, 0, 1, 3))
        hsT = np.ascontiguousarray(
            hs_c.transpose(2, 0, 1)             # [d, s, t]
            .reshape(4, 128, BL, T)
            .transpose(1, 0, 2, 3)
            .reshape(128, 4, BL * T))
        hl = np.asarray(hlens).reshape(-1)[seqs]
        maskhv = np.where(np.arange(T)[None, :] < hl[:, None],
                          0.0, -5e9).astype(f).reshape(1, -1)
        selv = np.zeros((B, 4 * BL), f)
        selwv = np.zeros((BL, 4 * BL), f)
        for j in range(BL):
            selv[BL * c + j, 4 * j + j] = 1.0
            selwv[j, 4 * j + j] = 1.0
        labv = np.full((nch * 128,), -1.0, f)
        lo = OS * c
        lb = ys_out_flat.astype(np.int64) - lo
        valid = (lb >= 0) & (lb < OS)
        labv[:nrow][valid] = lb[valid].astype(f)
        labv = labv.reshape(nch, 128).T.copy()  # [128, nch]

        in_maps.append({
            "hs_nat": hs_nat.astype(ml_dtypes.bfloat16),
            "hsT": hsT.astype(ml_dtypes.bfloat16),
            "eysT": eysT,
            "wih0pT": kpack(W_ih0[sl, :DUNITS].T, ml_dtypes.bfloat16),
            "x0bias": np.ascontiguousarray(bias0[sl][None]),
            "wencT": wencT.astype(ml_dtypes.bfloat16),
            "bencp": np.ascontiguousarray(bencpv.T),
            "wdecT": wdecT,
            "wattT": kpack(W_ih0[sl, DUNITS:].T, ml_dtypes.bfloat16),
            "whh0T": kpack(W_hh0[sl].T, ml_dtypes.bfloat16),
            "wih1T": kpack(W_ih1[sl].T, ml_dtypes.bfloat16),
            "whh1T": kpack(W_hh1[sl].T, ml_dtypes.bfloat16),
            "bias1": np.ascontiguousarray(bias1v[sl][None]).astype(ml_dtypes.bfloat16),
            "maskh": maskhv.astype(ml_dtypes.bfloat16),
            "sel": selv.astype(ml_dtypes.bfloat16),
            "selw": selwv.astype(ml_dtypes.bfloat16),
            "woutT": kpack(Wout[OS * c: OS * (c + 1)].T, ml_dtypes.bfloat16),
            "boutsl": np.ascontiguousarray(bout_v[OS * c: OS * (c + 1)][None]),
            "labels": labv,
            "ident": identv,
        })
    return in_maps


def _combine(results, steps):
    """Merge per-core (m, S, lab) partials into (loss, acc, ppl)."""
    nrow = steps * B
    ms, ss, labs = [], [], []
    for r in results:
        st = r["out_stats"]                     # [128, nch, 3]
        ms.append(st[:, :, 0].T.reshape(-1)[:nrow])
        ss.append(st[:, :, 1].T.reshape(-1)[:nrow])
        labs.append(st[:, :, 2].T.reshape(-1)[:nrow])
    m = np.stack(ms)
    s = np.stack(ss)
    lab = np.stack(labs)
    gmax = m.max(axis=0)
    gsum = (s.astype(np.float64)
            * np.exp(m.astype(np.float64) - gmax[None])).sum(axis=0)
    lablogit = lab.sum(axis=0)
    nll = gmax.astype(np.float64) + np.log(gsum) - lablogit
    match = (lab == gmax[None]).any(axis=0)
    loss = np.float32(nll.mean() * L)
    acc = np.float32(match.mean())
    ppl = np.float32(np.exp(np.float64(loss) / B))
    return loss, acc, ppl


def kernel(**inputs):
    steps = S
    in_maps = _prep_inputs(steps=steps, **inputs)
    if steps not in _BUILD_CACHE:
        _BUILD_CACHE[steps] = build(steps)
    nc = _BUILD_CACHE[steps]
    res = bass_utils.run_bass_kernel_spmd(
        nc, in_maps, core_ids=list(range(NC)))
    return _combine(res.results, steps)
